# revision 12
# baseline (speedup 1.0000x reference)
"""Trainium2 Bass kernel for nn_BaselineAttn (LoRA QKV + ALiBi causal attention).

Sharding: 8 cores SPMD, no collectives. Core c = (b, g): batch b = c // 4,
head group g = c % 4 handling heads [g, 4+g, 8+g, 12+g] in slots 0..3.

Host prep: LoRA folded into weights (W' = W + 2 A@B); x and weights
pre-transposed/sliced per core; partial f16 outputs summed on host.

Device design (fp16 operands, fp32 PSUM):
  - attention in the S^T (key-major) orientation: S^T tile = kT.T @ qT.
  - ALiBi weight exp(-slope*k) is split: exp(-slope*p) (p = partition, via
    per-SLOT activation bias) times c(kt) = exp(-slope*128*kt) folded into
    the V columns and the denominator columns (cfac, host data).  The bias
    is then identical for every key tile of a slot, so TWO adjacent full
    S^T tiles land in one 2-bank PSUM tile and share ONE exp activation.
  - vext = [c*v | c*ones]: the OT matmul's free-size cost doesn't depend
    on out partitions, so columns 64:128 replicate the softmax denominator
    onto partitions 64:128 pre-broadcast (no DRAM bounce).  v scaling by c
    happens during PSUM copy-out as a tensor_mul against vext's own
    c-columns.  Normalize: copy denom to SBUF partitions 0:64, reciprocal
    at base partition 0, one cross-offset DVE mul.
  - p is f16 (subnormal floor ~e^-17): key tiles with slope*128*kt > ~16
    are exactly 0 and are skipped: SNKT = [1, 2, 8, 16] per-slot caps.
  - software pipelining: per attention unit (diag single or full-tile
    pair) the previous unit's OTs are emitted after the current S/exp with
    one QKV/proj filler matmul in between, so the PE never waits for the
    Act engine's exp; leftover QKV drains in bulk at chunk boundaries.
"""

import math
from collections import deque

import numpy as np

E = 1024
H = 16
DH = 64
T = 2048
BATCH = 2
LORA_S = 2.0
NKT = T // 128          # 16 key tiles of 128
SNKT = [1, 2, 8, 16]    # per-slot key-tile caps (max over cores per slot)
NQC = 4                 # q chunks of 512

_NC_CACHE = {}


def _slopes():
    start = 2 ** (-2 ** (-(math.log2(H) - 3)))
    return np.array([start * start**i for i in range(H)], dtype=np.float64)


def _smin(tt):
    """Lowest slot that still needs key-tile tt."""
    for s in range(4):
        if tt < SNKT[s]:
            return s
    return 4


def _build_nc():
    """Build the single SPMD Bass program (shared by all 8 cores)."""
    if "nc" in _NC_CACHE:
        return _NC_CACHE["nc"]

    from concourse.bacc import Bacc
    import concourse.tile as tile
    from concourse import mybir

    f16 = mybir.dt.float16
    f32 = mybir.dt.float32
    EXP = mybir.ActivationFunctionType.Exp

    nc = Bacc()

    xT_d = nc.dram_tensor("xT", [E, T], f16, kind="ExternalInput")
    wqkv_d = nc.dram_tensor("wqkvT", [E, 768], f16, kind="ExternalInput")
    wp_d = nc.dram_tensor("wpT", [256, E], f16, kind="ExternalInput")
    bias_d = nc.dram_tensor("expbias", [128, 4], f32, kind="ExternalInput")
    mask_d = nc.dram_tensor("masks", [128, 4 * 512], f16, kind="ExternalInput")
    cfac_d = nc.dram_tensor("cfac", [NKT, 256], f16, kind="ExternalInput")
    out_d = nc.dram_tensor("outp", [T, E], f16, kind="ExternalOutput")

    with tile.TileContext(nc) as tc:
        with (
            tc.tile_pool(name="persist", bufs=1) as pp,
            tc.tile_pool(name="ptpool", bufs=6) as ptp,
            tc.tile_pool(name="onorm", bufs=4) as onp,
            tc.tile_pool(name="rpool", bufs=4) as rp,
            tc.tile_pool(name="outsb", bufs=6) as osp,
            tc.tile_pool(name="stps", bufs=2, space="PSUM") as stps,
            tc.tile_pool(name="otps", bufs=2, space="PSUM") as otps,
            tc.tile_pool(name="fillps", bufs=2, space="PSUM") as fillps,
        ):
            # ---- input DMA, ordered for earliest PE start: w_q cols, then x
            # chunk 0, then the rest.
            wqkv = [pp.tile([128, 768], f16, name=f"wqkv{kt}") for kt in range(8)]
            xT = [pp.tile([128, T], f16, name=f"xT{kt}") for kt in range(8)]

            def ddma(i, out, in_):
                (nc.sync if i % 2 else nc.scalar).dma_start(out=out, in_=in_)

            for kt in range(8):
                ddma(kt, wqkv[kt][:, 0:256], wqkv_d[kt * 128:(kt + 1) * 128, 0:256])
            for kt in range(8):
                ddma(kt, xT[kt][:, 0:512], xT_d[kt * 128:(kt + 1) * 128, 0:512])
            bias_sb = pp.tile([128, 4], f32, name="bias")
            nc.sync.dma_start(out=bias_sb, in_=bias_d[:, :])
            for kt in range(8):
                ddma(kt, wqkv[kt][:, 256:768], wqkv_d[kt * 128:(kt + 1) * 128, 256:768])
            mask_sb = pp.tile([128, 4 * 512], f16, name="mask")
            nc.scalar.dma_start(out=mask_sb, in_=mask_d[:, :])
            # vext[tt]: [128 keys, slot, 128]: cols 0:64 = c*v, 64:128 = c.
            vext = []
            for tt in range(NKT):
                v_t = pp.tile([128, 4, 128], f16, name=f"vext{tt}")
                nc.sync.dma_start(
                    out=v_t[:, :, 64:128],
                    in_=cfac_d[tt:tt + 1, :].to_broadcast([128, 256]))
                vext.append(v_t)
            for ncu in range(1, NQC):
                for kt in range(8):
                    ddma(kt, xT[kt][:, ncu * 512:(ncu + 1) * 512],
                         xT_d[kt * 128:(kt + 1) * 128, ncu * 512:(ncu + 1) * 512])
            wp = []
            for pt in range(2):
                wp_t = pp.tile([128, E], f16, name=f"wp{pt}")
                nc.scalar.dma_start(out=wp_t, in_=wp_d[pt * 128:(pt + 1) * 128, :])
                wp.append(wp_t)

            # q^T / k^T: per (p-tile, chunk) tiles [128, 512].
            # kT p-tile 0 (slots 0,1) only needs k < 256: chunk 0 only.
            qT = [[pp.tile([128, 512], f16, name=f"qT{p}_{ncu}") for ncu in range(NQC)]
                  for p in range(2)]
            kT = [[pp.tile([128, 512], f16, name=f"kT{p}_{ncu}")
                   if (p == 1 or ncu < 1) else None for ncu in range(NQC)]
                  for p in range(2)]

            state = {"ncopy": 0, "nosb": 0}

            # ---------- filler step groups (each group = one PSUM acc) ----------
            def qk_acc_group(wofs, mt, ncu):
                dst = (qT, kT)[wofs // 256]
                nw = 256 if (wofs == 256 and mt == 0) else 512
                cell = {}
                steps = []
                for kt in range(8):
                    def mm(kt=kt):
                        if kt == 0:
                            cell["acc"] = fillps.tile(
                                [128, 512], f32, tag="facc",
                                name=f"qkacc{wofs}_{mt}_{ncu}")
                        nc.tensor.matmul(
                            cell["acc"][:, 0:nw],
                            wqkv[kt][:, wofs + mt * 128:wofs + (mt + 1) * 128],
                            xT[kt][:, ncu * 512:ncu * 512 + nw],
                            start=(kt == 0), stop=(kt == 7),
                        )
                    steps.append(mm)

                def cp():
                    state["ncopy"] += 1
                    if state["ncopy"] % 2:
                        nc.scalar.copy(out=dst[mt][ncu][:, 0:nw],
                                       in_=cell["acc"][:, 0:nw])
                    else:
                        nc.vector.tensor_copy(out=dst[mt][ncu][:, 0:nw],
                                              in_=cell["acc"][:, 0:nw])
                steps.append(cp)
                return steps

            def v_acc_group(tt):
                s0 = _smin(tt)
                nw = (4 - s0) * 64
                cell = {}
                steps = []
                for kt in range(8):
                    def mm(kt=kt):
                        if kt == 0:
                            cell["acc"] = fillps.tile([128, 512], f32, tag="facc",
                                                      name=f"vacc{tt}")
                        nc.tensor.matmul(
                            cell["acc"][:, 0:nw],
                            xT[kt][:, tt * 128:(tt + 1) * 128],
                            wqkv[kt][:, 512 + s0 * 64:768],
                            start=(kt == 0), stop=(kt == 7),
                        )
                    steps.append(mm)

                def cp():
                    # scale v by c(tt, s) during copy-out: multiply the PSUM
                    # acc against vext's own c-columns (same value repeated
                    # across the 64-wide block).  Act has no tensor_tensor,
                    # Pool can't read PSUM: DVE only.
                    nc.vector.tensor_mul(
                        out=vext[tt][:, s0:4, 0:64],
                        in0=cell["acc"][:, 0:nw].rearrange("p (s d) -> p s d", d=64),
                        in1=vext[tt][:, s0:4, 64:128],
                    )
                steps.append(cp)
                return steps

            def chunk_groups(ncu):
                groups = []
                for mt in range(2):
                    groups.append(qk_acc_group(0, mt, ncu))
                for mt in range(2):
                    if kT[mt][ncu] is not None:
                        groups.append(qk_acc_group(256, mt, ncu))
                for tt in range(4 * ncu, 4 * ncu + 4):
                    groups.append(v_acc_group(tt))
                return groups

            def proj_groups(qc, on_tiles):
                groups = []
                for tloc in range(4):
                    tt = qc * 4 + tloc
                    for ech in range(2):
                        cell = {}

                        def mm1(tt=tt, ech=ech, cell=cell):
                            cell["acc"] = fillps.tile([128, 512], f32, tag="facc",
                                                      name=f"pacc_{tt}_{ech}")
                            nc.tensor.matmul(
                                cell["acc"],
                                on_tiles[1][:, (tt % 4) * 128:(tt % 4 + 1) * 128],
                                wp[1][:, ech * 512:(ech + 1) * 512],
                                start=True, stop=False,
                            )

                        def mm0(tt=tt, ech=ech, cell=cell):
                            nc.tensor.matmul(
                                cell["acc"],
                                on_tiles[0][:, (tt % 4) * 128:(tt % 4 + 1) * 128],
                                wp[0][:, ech * 512:(ech + 1) * 512],
                                start=False, stop=True,
                            )

                        def outstep(tt=tt, ech=ech, cell=cell):
                            osb = osp.tile([128, 512], f16, tag="osb",
                                           name=f"osb_{tt}_{ech}")
                            state["nosb"] += 1
                            if state["nosb"] % 2:
                                nc.vector.tensor_copy(out=osb, in_=cell["acc"])
                            else:
                                nc.scalar.copy(out=osb, in_=cell["acc"])
                            nc.sync.dma_start(
                                out=out_d[tt * 128:(tt + 1) * 128,
                                          ech * 512:(ech + 1) * 512],
                                in_=osb)
                        groups.append([mm1, mm0, outstep])
                return groups

            # ---------- filler scheduler ----------
            due = {ncu: deque(chunk_groups(ncu)) for ncu in range(1, NQC)}
            fillers = deque()          # proj groups (no deadline)
            cur = {"steps": None}      # partially-consumed group

            def _next_group(qc):
                nxt = due.get(qc + 1)
                if nxt:
                    return nxt.popleft()
                if fillers:
                    return fillers.popleft()
                return None

            def drain_one(qc):
                if cur["steps"]:
                    cur["steps"].pop(0)()
                    if not cur["steps"]:
                        cur["steps"] = None
                    return
                g = _next_group(qc)
                if g is None:
                    return
                g.pop(0)()
                cur["steps"] = g if g else None

            def finish_cur():
                while cur["steps"]:
                    cur["steps"].pop(0)()
                    if not cur["steps"]:
                        cur["steps"] = None

            def drain_all(q):
                finish_cur()
                while q:
                    for st_ in q.popleft():
                        st_()

            # chunk 0 QKV runs eagerly (attention q0 needs all of it).
            for g in chunk_groups(0):
                for st_ in g:
                    st_()

            # ---------- attention ----------
            pending = []

            def flush():
                for f in pending:
                    f()
                pending.clear()

            nmask = 0
            for qc in range(NQC):
                on_tiles = [onp.tile([128, 512], f16, tag="on", name=f"on_{qc}_{p}")
                            for p in range(2)]
                for pair in (1, 0):
                    for s in (2 * pair + 1, 2 * pair):
                        nkt = min(SNKT[s], 4 * qc + 4)
                        pt_i = pair
                        r0 = 64 * (s % 2)
                        diag = [kt for kt in range(nkt) if kt >= 4 * qc]
                        full = [kt for kt in range(nkt) if kt < 4 * qc]
                        units = [("single", (kt,)) for kt in diag]
                        i = 0
                        while i + 1 < len(full):
                            units.append(("pair", (full[i], full[i + 1])))
                            i += 2
                        if i < len(full):
                            units.append(("single", (full[i],)))
                        ot = otps.tile([128, 512], f32, tag="ot",
                                       name=f"ot_{qc}_{s}")
                        n_units = len(units)
                        with nc.named_scope(f"attn_q{qc}_s{s}"):
                            for ui, (kind, kts) in enumerate(units):
                                first = (ui == 0)
                                last = (ui == n_units - 1)
                                st = stps.tile([128, 1024], f32, tag="st2",
                                               name=f"st_{qc}_{s}_{kts[0]}")
                                p_t = ptp.tile([128, 1024], f16, tag="pt",
                                               name=f"pt_{qc}_{s}_{kts[0]}")
                                if kind == "single":
                                    kt = kts[0]
                                    j0 = (kt - 4 * qc) * 128 if kt >= 4 * qc else 0
                                    nc.tensor.matmul(
                                        st[:, j0:512],
                                        kT[pt_i][kt // 4][r0:r0 + 64,
                                                          (kt % 4) * 128:(kt % 4 + 1) * 128],
                                        qT[pt_i][qc][r0:r0 + 64, j0:512],
                                        start=True, stop=True,
                                    )
                                    nc.scalar.activation(
                                        out=p_t[:, j0:512], in_=st[:, j0:512],
                                        func=EXP, bias=bias_sb[:, s:s + 1],
                                        scale=0.125,
                                    )
                                    if kt >= 4 * qc:
                                        m = kt - 4 * qc
                                        nmask += 1
                                        nc.gpsimd.tensor_mul(
                                            out=p_t[:, j0:512],
                                            in0=p_t[:, j0:512],
                                            in1=mask_sb[:, m * 512 + j0:(m + 1) * 512],
                                        )

                                    def emit_ot(kt=kt, j0=j0, p_tt=p_t, ot_t=ot,
                                                s=s, first=first, last=last):
                                        nc.tensor.matmul(
                                            ot_t[:, j0:512],
                                            vext[kt][:, s, :],
                                            p_tt[:, j0:512],
                                            start=first, stop=last,
                                        )
                                else:
                                    for half, kt in enumerate(kts):
                                        nc.tensor.matmul(
                                            st[:, half * 512:(half + 1) * 512],
                                            kT[pt_i][kt // 4][r0:r0 + 64,
                                                              (kt % 4) * 128:(kt % 4 + 1) * 128],
                                            qT[pt_i][qc][r0:r0 + 64, 0:512],
                                            start=True, stop=True,
                                        )
                                    nc.scalar.activation(
                                        out=p_t[:, 0:1024], in_=st[:, 0:1024],
                                        func=EXP, bias=bias_sb[:, s:s + 1],
                                        scale=0.125,
                                    )

                                    def emit_ot(kts=kts, p_tt=p_t, ot_t=ot, s=s,
                                                first=first, last=last):
                                        for half, kt in enumerate(kts):
                                            nc.tensor.matmul(
                                                ot_t[:, 0:512],
                                                vext[kt][:, s, :],
                                                p_tt[:, half * 512:(half + 1) * 512],
                                                start=(first and half == 0),
                                                stop=(last and half == 1),
                                            )
                                pend_new = [emit_ot]
                                if last:
                                    def emit_norm(ot_t=ot, s=s, qc=qc, pair=pair,
                                                  r0=r0, on=on_tiles):
                                        den = rp.tile([128, 512], f32, tag="den",
                                                      name=f"den_{qc}_{s}")
                                        rec = rp.tile([128, 512], f32, tag="rec",
                                                      name=f"rec_{qc}_{s}")
                                        nc.vector.tensor_copy(out=den[0:64, :],
                                                              in_=ot_t[64:128, :])
                                        nc.vector.reciprocal_approx_fast(
                                            out=rec[0:64, :], in_=den[0:64, :])
                                        nc.vector.tensor_mul(
                                            out=on[pair][r0:r0 + 64, :],
                                            in0=ot_t[0:64, :],
                                            in1=rec[0:64, :],
                                        )
                                    pend_new.append(emit_norm)
                                drain_one(qc)
                                flush()
                                pending.extend(pend_new)
                flush()
                # boundary: finish proj leftovers, queue this chunk's proj,
                # then fully emit the next chunk's QKV.
                finish_cur()
                while fillers:
                    for st_ in fillers.popleft():
                        st_()
                fillers.extend(proj_groups(qc, on_tiles))
                if qc + 1 in due:
                    drain_all(due[qc + 1])
            finish_cur()
            while fillers:
                for st_ in fillers.popleft():
                    st_()

    nc.finalize()
    _NC_CACHE["nc"] = nc
    return nc


def _prep_core_inputs(x, Wq, Aq, Bq, Wk, Ak, Bk, Wv, Av, Bv, Wp):
    """Host-side prep: LoRA fold, transposes, per-core slices."""
    slopes = _slopes()
    wq_m = Wq.astype(np.float64) + LORA_S * (Aq.astype(np.float64) @ Bq.astype(np.float64))
    wk_m = Wk.astype(np.float64) + LORA_S * (Ak.astype(np.float64) @ Bk.astype(np.float64))
    wv_m = Wv.astype(np.float64) + LORA_S * (Av.astype(np.float64) @ Bv.astype(np.float64))

    # mask_m[p, j] = 1 if (m*128 + p) <= j else 0   (j in 0..511)
    p_i = np.arange(128)[:, None]
    j_i = np.arange(512)[None, :]
    masks = np.ascontiguousarray(np.concatenate(
        [((m * 128 + p_i) <= j_i).astype(np.float16) for m in range(4)], axis=1))

    in_maps = []
    for c in range(8):
        b, g = divmod(c, 4)
        heads = [g, 4 + g, 8 + g, 12 + g]
        rows = np.concatenate([np.arange(h * DH, (h + 1) * DH) for h in heads])
        xT = np.ascontiguousarray(x[b].T.astype(np.float16))
        wqkvT = np.ascontiguousarray(np.concatenate(
            [wq_m[rows, :].T, wk_m[rows, :].T, wv_m[rows, :].T],
            axis=1).astype(np.float16))
        wpT = np.ascontiguousarray(Wp[:, rows].T.astype(np.float16))
        bias = np.zeros((128, 4), dtype=np.float32)
        cfac = np.zeros((NKT, 4, 64), dtype=np.float32)
        for s, h in enumerate(heads):
            bias[:, s] = -slopes[h] * np.arange(128)
            for kt in range(NKT):
                cfac[kt, s, :] = math.exp(-slopes[h] * 128 * kt)
        in_maps.append({
            "xT": xT, "wqkvT": wqkvT, "wpT": wpT,
            "expbias": bias, "masks": masks,
            "cfac": np.ascontiguousarray(
                cfac.reshape(NKT, 256).astype(np.float16)),
        })
    return in_maps


def _run(in_maps, trace=False, **kw):
    from concourse.bass_utils import run_bass_kernel_spmd
    nc = _build_nc()
    return run_bass_kernel_spmd(nc, in_maps, core_ids=list(range(8)), trace=trace, **kw)


def kernel(x, Wq, Aq, Bq, Wk, Ak, Bk, Wv, Av, Bv, Wp):
    in_maps = _prep_core_inputs(x, Wq, Aq, Bq, Wk, Ak, Bk, Wv, Av, Bv, Wp)
    res = _run(in_maps)
    out = np.zeros((BATCH, T, E), dtype=np.float32)
    for c in range(8):
        out[c // 4] += res.results[c]["outp"].astype(np.float32)
    return out
